# revision 23
# baseline (speedup 1.0000x reference)
"""Trainium2 Bass kernel for nn_Bottleneck_5669356834470 (ResNet bottleneck
with an involution middle layer) — v4.

Sharding: data-parallel over batch. 16 samples / 8 cores = 2 samples/core.

Key changes vs v3:
  * tap accumulation split across engines: channels 0-3 of each (group,
    half) accumulate on the (otherwise idle) PE via identity-stationary
    matmuls into 4 PSUM banks; channels 4-7 keep the DVE add path.  DVE
    only multiplies for the PE channels, cutting its tap cost ~25%.
  * inv_c1 PSUM evac moved DVE -> ACT (PSUM-input DVE ops run 1x).
  * conv1 / conv3 matmuls batched per stationary operand (2-chunk groups)
    to stop LDWEIGHTS thrash; PSUM bufs rebalanced 2+2+4 (ps/psw/tap-acc).
  * x load split into half-sample DMAs; w2 DRAM staging issued per sample
    so the first tap range lands sooner.
"""

import sys

sys.path.insert(0, "/opt/trn_rl_repo")

import numpy as np
import ml_dtypes

BF16 = ml_dtypes.bfloat16

S = 2            # samples per core
N_CORES = 8
CIN = 256
CMID = 64
G = 4            # involution groups
GC = 16          # channels per group
C8 = 8           # channels per (group, half)
NH = 2           # channel halves per group
KS = 7           # involution kernel size
KK = KS * KS     # 49
R = 16           # dyn-weight bottleneck channels
H = W = 56
HW = H * W       # 3136
RH = 7           # output rows per partition chunk
MCH = 8          # row chunks per (s, g, h)
NP = 128         # partitions = S*G*NH*MCH
NW = RH * W      # 392: matmul / staging chunk (conv1 side)
NWP = RH * 62    # 434: row-padded pixel chunk (w2 / tap side)
NCH = 8          # spatial chunks per sample
HR = 14          # halo rows stored per chunk (13 valid + 1 zero pad)
WP = 62          # padded row width
RUN = 6 * WP + W     # 428: contiguous tap run (7 rows incl inter-row pads)
PAD = 3 * W      # 168: zero margin per sample in out1d
SPX = PAD + HW + PAD   # 3472: out1d pixels per sample
XUF = C8 * HR * W    # 6272 free elems per XU partition
XHF = C8 * HR * WP   # 6944 free elems per XH partition
W2F = KK * NWP       # 21266 free elems per W2T partition (row-padded)
ACCF = C8 * NWP      # 3472 acc free elems per partition (row-padded)
ACCC = C8 * NW       # 3136 compact acc free elems per partition
EPS = 1e-5
CP = 6           # channels per (g,h) accumulated on the PE (PSUM banks)
CV = C8 - CP     # channels per (g,h) accumulated on DVE

# Tap multiplies all on DVE (GpSimd tensor ops contend for SBUF and degrade
# DVE throughput ~4x, measured on HW).  Taps ordered by k so they can start
# as soon as the first k-range of the weight gather lands.

_CACHE = {}


def _p(s, g, h, m):
    return ((s * G + g) * NH + h) * MCH + m


def _ap(tile_ap, off, dims):
    """Raw strided AP on a tile's underlying tensor. dims=[(step,count),...]
    in elements; for SBUF the partition stride is ap[0][0] of the base AP."""
    import bass_rust

    return bass_rust.AP(tile_ap.tensor, tile_ap.offset + off, [list(d) for d in dims])


def build_module():
    if "nc" in _CACHE:
        return _CACHE["nc"]
    import concourse.bacc as bacc
    import concourse.mybir as mybir
    import concourse.tile as tile

    dt = mybir.dt
    AF = mybir.ActivationFunctionType
    ALU = mybir.AluOpType

    nc = bacc.Bacc("TRN2", debug=False, num_devices=N_CORES)

    # ---- DRAM I/O ----------------------------------------------------------
    xin = nc.dram_tensor("xin", [S, CIN, HW], dt.float32, kind="ExternalInput")
    w1t = nc.dram_tensor("w1t", [2, 128, CMID], dt.bfloat16, kind="ExternalInput")
    b1 = nc.dram_tensor("b1", [CMID, 1], dt.float32, kind="ExternalInput")
    c1t = nc.dram_tensor("c1t", [CMID, R], dt.bfloat16, kind="ExternalInput")
    bi = nc.dram_tensor("bi", [R, 1], dt.float32, kind="ExternalInput")
    c2t = nc.dram_tensor("c2t", [R, G * KK], dt.bfloat16, kind="ExternalInput")
    b2ca = nc.dram_tensor("b2ca", [128, 1], dt.float32, kind="ExternalInput")
    b2cb = nc.dram_tensor("b2cb", [68, 1], dt.float32, kind="ExternalInput")
    s2v = nc.dram_tensor("s2v", [CMID, 1], dt.float32, kind="ExternalInput")
    b2v = nc.dram_tensor("b2v", [CMID, 1], dt.float32, kind="ExternalInput")
    w3t = nc.dram_tensor("w3t", [2, CMID, 128], dt.bfloat16, kind="ExternalInput")
    b3 = nc.dram_tensor("b3", [128, 2], dt.float32, kind="ExternalInput")
    ident = nc.dram_tensor("ident", [128, 128], dt.bfloat16, kind="ExternalInput")
    out = nc.dram_tensor("out", [S, CIN, HW], dt.bfloat16, kind="ExternalOutput")

    with tile.TileContext(nc) as tc:
        with (
            tc.tile_pool(name="consts", bufs=1) as cpool,
            tc.tile_pool(name="big", bufs=1) as bpool,
            tc.tile_pool(name="zst", bufs=4) as zpool,
            tc.tile_pool(name="rst", bufs=2) as rpool,
            tc.tile_pool(name="ob", bufs=2) as opool,
            tc.tile_pool(name="dstage", bufs=1, space="DRAM") as dpool,
        ):
            # ---- constants -> SBUF ----------------------------------------
            w1t_sb = cpool.tile([128, 2 * CMID], dt.bfloat16, tag="w1t")
            nc.sync.dma_start(
                out=w1t_sb[:, :].rearrange("p (k c) -> p k c", k=2),
                in_=w1t.ap().rearrange("k p c -> p k c"),
            )
            b1_sb = cpool.tile([CMID, 1], dt.float32, tag="b1")
            nc.sync.dma_start(out=b1_sb[:, :], in_=b1.ap())
            c1t_sb = cpool.tile([CMID, R], dt.bfloat16, tag="c1t")
            nc.sync.dma_start(out=c1t_sb[:, :], in_=c1t.ap())
            bi_sb = cpool.tile([R, 1], dt.float32, tag="bi")
            nc.sync.dma_start(out=bi_sb[:, :], in_=bi.ap())
            c2t_sb = cpool.tile([R, G * KK], dt.bfloat16, tag="c2t")
            nc.sync.dma_start(out=c2t_sb[:, :], in_=c2t.ap())
            b2ca_sb = cpool.tile([128, 1], dt.float32, tag="b2ca")
            nc.sync.dma_start(out=b2ca_sb[:, :], in_=b2ca.ap())
            b2cb_sb = cpool.tile([68, 1], dt.float32, tag="b2cb")
            nc.sync.dma_start(out=b2cb_sb[:, :], in_=b2cb.ap())
            s2v_sb = cpool.tile([CMID, 1], dt.float32, tag="s2v")
            nc.sync.dma_start(out=s2v_sb[:, :], in_=s2v.ap())
            b2v_sb = cpool.tile([CMID, 1], dt.float32, tag="b2v")
            nc.sync.dma_start(out=b2v_sb[:, :], in_=b2v.ap())
            w3t_sb = cpool.tile([CMID, 2 * 128], dt.bfloat16, tag="w3t")
            nc.sync.dma_start(
                out=w3t_sb[:, :].rearrange("p (k c) -> p k c", k=2),
                in_=w3t.ap().rearrange("k p c -> p k c"),
            )
            b3_sb = cpool.tile([128, 2], dt.float32, tag="b3")
            nc.sync.dma_start(out=b3_sb[:, :], in_=b3.ap())
            id_sb = cpool.tile([128, 128], dt.bfloat16, tag="ident")
            nc.sync.dma_start(out=id_sb[:, :], in_=ident.ap())

            # ---- big SBUF tiles -------------------------------------------
            xbf = bpool.tile([128, S * 2 * HW], dt.bfloat16, tag="xbf")
            # sized for its out2 reuse (62-padded layout needs S*MCH*NWP=6944)
            out1 = bpool.tile([CMID, S * MCH * NWP], dt.bfloat16, tag="out1")
            w2a = bpool.tile([128, S * MCH * NWP], dt.bfloat16, tag="w2a")
            w2b = bpool.tile([68, S * MCH * NWP], dt.bfloat16, tag="w2b")
            xu = bpool.tile([NP, XUF], dt.bfloat16, tag="xu")
            xh = bpool.tile([NP, XHF], dt.bfloat16, tag="xh")
            xh2 = bpool.tile([NP, XHF], dt.bfloat16, tag="xh2")
            w2t = bpool.tile([NP, W2F], dt.bfloat16, tag="w2t")
            acc2 = bpool.tile([NP, 2 * ACCF], dt.bfloat16, tag="acc2")
            tmp2 = [
                bpool.tile([NP, 2 * ACCF], dt.bfloat16, tag=f"tmp2_{i}", name=f"tmp2_{i}")
                for i in range(2)
            ]
            out2 = out1  # out1 is dead after the xu gathers; reuse for out2
            zt = bpool.tile([CMID, PAD], dt.bfloat16, tag="zt")
            zst2 = [
                bpool.tile([R, NWP], dt.bfloat16, tag=f"zst{i}", name=f"zst{i}")
                for i in range(2)
            ]

            xbf_v = xbf[:, :].rearrange("p (s k f) -> p s k f", s=S, k=2)
            xu_v = xu[:, :].rearrange("p (c r w) -> p c r w", c=C8, r=HR, w=W)
            xh_v = xh[:, :].rearrange("p (c r w) -> p c r w", c=C8, r=HR, w=WP)
            xh2_v = xh2[:, :].rearrange("p (c r w) -> p c r w", c=C8, r=HR, w=WP)
            # flat views for the long-run tap ops
            xh_f = xh[:, :].rearrange("p (c f) -> p c f", c=C8)
            xh2_f = xh2[:, :].rearrange("p (c f) -> p c f", c=C8)
            P_XH = xh[:, :].ap[0][0]
            P_XH2 = xh2[:, :].ap[0][0]
            P_A2 = acc2[:, :].ap[0][0]
            P_T2 = [t[:, :].ap[0][0] for t in tmp2]

            # ---- DRAM staging ---------------------------------------------
            # w2d is (s,m)-major so the w2t gather per partition is one
            # contiguous multi-KB run (DRAM-sequential, not 111KB strides).
            out1d = dpool.tile([CMID, S * SPX], dt.bfloat16, tag="out1d")
            w2d = dpool.tile([S * MCH, G * KK * NWP], dt.bfloat16, tag="w2d")
            accd = dpool.tile([CMID, S * MCH * NWP], dt.bfloat16, tag="accd")

            # ---- memzeros (pads for halo tensors) -------------------------
            # all on DVE (4x memset) so the ACT queue is free for the conv1
            # evacuations from the very start
            nc.vector.memset(xu[:, :], 0.0)
            nc.vector.memset(xh[:, :], 0.0)
            nc.vector.memset(xh2[:, :], 0.0)
            nc.vector.memset(zt[:, :], 0.0)
            nc.vector.memset(acc2[:, :], 0.0)
            for z in zst2:
                nc.vector.memset(z[:, :], 0.0)

            # zero margins of out1d so halo gathers read zeros off the edges
            for s in range(S):
                nc.sync.dma_start(
                    out=out1d[:, s * SPX : s * SPX + PAD], in_=zt[:, :]
                )
                nc.sync.dma_start(
                    out=out1d[:, s * SPX + PAD + HW : (s + 1) * SPX], in_=zt[:, :]
                )

            # ---- x load (f32 -> bf16 cast; SWDGE on gpsimd) ---------------
            # split in half-sample pieces so conv1 can start on the first
            # chunks while the rest streams in
            HWH = HW // 2
            for s in range(S):
                for hh in range(2):
                    for kc in range(2):
                        nc.gpsimd.dma_start(
                            out=xbf_v[:, s, kc, hh * HWH : (hh + 1) * HWH],
                            in_=xin.ap()[
                                s, kc * 128 : (kc + 1) * 128, hh * HWH : (hh + 1) * HWH
                            ],
                        )

            # ---- per-sample front end: conv1 / inv_c1 / inv_c2 ------------
            w1t_v = w1t_sb[:, :].rearrange("p (k c) -> p k c", k=2)
            o1d_ap = out1d[:, :]
            w2d_ap = w2d[:, :]
            xu_ap = xu[:, :]
            w2t_ap = w2t[:, :]
            D1 = o1d_ap.ap[0][0]
            D2 = w2d_ap.ap[0][0]
            P_XU = xu_ap.ap[0][0]
            P_W2T = w2t_ap.ap[0][0]

            pfe_cm = tc.tile_pool(name="psum_fe", bufs=2, space="PSUM")
            pfe = pfe_cm.__enter__()
            for s in range(S):
                for n0 in range(0, NCH, 2):
                    # conv1 (256->64) + BN1 + ReLU   [ACT evac]
                    # 2-chunk batches per stationary operand: LDWEIGHTS for
                    # w1t[kc] loads once per pair instead of per chunk.
                    pss = []
                    for kc in range(2):
                        for dn in range(2):
                            n = n0 + dn
                            if kc == 0:
                                pss.append(
                                    pfe.tile(
                                        [128, NW], dt.float32, tag="ps", bufs=2,
                                        name=f"ps_{s}_{n}",
                                    )
                                )
                            nc.tensor.matmul(
                                pss[dn][:CMID, :],
                                w1t_v[:, kc, :],
                                xbf_v[:, s, kc, n * NW : (n + 1) * NW],
                                start=(kc == 0),
                                stop=(kc == 1),
                            )
                    for dn in range(2):
                        n = n0 + dn
                        sl = slice(s * HW + n * NW, s * HW + (n + 1) * NW)
                        nc.scalar.activation(
                            out1[:, sl], pss[dn][:CMID, :], AF.Relu, bias=b1_sb[:, 0:1]
                        )
                    for dn in range(2):
                        n = n0 + dn
                        sl = slice(s * HW + n * NW, s * HW + (n + 1) * NW)
                        # inv_c1 (64->16) + BN + ReLU    [ACT evac]
                        # zst is row-padded to 62-wide rows; the pad columns
                        # keep stale (finite) values which flow through inv_c2
                        # into pad weight columns that only multiply zeros.
                        ps1 = pfe.tile([128, NW], dt.float32, tag="ps", bufs=2)
                        nc.tensor.matmul(
                            ps1[:R, :], c1t_sb[:, :], out1[:, sl], start=True, stop=True
                        )
                        zst = zst2[n % 2]
                        zst_v = zst[:, :].rearrange("p (r w) -> p r w", r=RH, w=WP)
                        nc.scalar.activation(
                            zst_v[:, :, 0:W],
                            ps1[:R, :].rearrange("p (r w) -> p r w", r=RH, w=W),
                            AF.Relu,
                            bias=bi_sb[:, 0:1],
                        )
                        # inv_c2 (16->196) + bias        [DVE + ACT evacs]
                        psa = pfe.tile([128, NWP], dt.float32, tag="psw", bufs=2)
                        psb = pfe.tile([128, NWP], dt.float32, tag="psw", bufs=2)
                        nc.tensor.matmul(
                            psa[:, :], c2t_sb[:, 0:128], zst[:, :], start=True, stop=True
                        )
                        nc.tensor.matmul(
                            psb[:68, :], c2t_sb[:, 128:196], zst[:, :],
                            start=True, stop=True,
                        )
                        nsl = slice((s * MCH + n) * NWP, (s * MCH + n + 1) * NWP)
                        nc.vector.tensor_scalar(
                            w2a[:, nsl], psa[:, :], b2ca_sb[:, 0:1], None, op0=ALU.add
                        )
                        nc.scalar.activation(
                            w2b[:, nsl], psb[:68, :], AF.Identity, bias=b2cb_sb[:, 0:1]
                        )

                    # stage out1 through DRAM as each 2-chunk batch finishes
                    nc.sync.dma_start(
                        out=out1d[
                            :,
                            s * SPX + PAD + n0 * NW : s * SPX + PAD + (n0 + 2) * NW,
                        ],
                        in_=out1[:, s * HW + n0 * NW : s * HW + (n0 + 2) * NW],
                    )

                # w2 -> (s,m)-major DRAM per sample (dst runs gk-seq 868B)
                nc.sync.dma_start(
                    out=_ap(w2d_ap, s * MCH * D2, [(NWP, 128), (D2, MCH), (1, NWP)]),
                    in_=_ap(
                        w2a[:, :],
                        s * MCH * NWP,
                        [(w2a[:, :].ap[0][0], 128), (NWP, MCH), (1, NWP)],
                    ),
                )
                nc.sync.dma_start(
                    out=_ap(
                        w2d_ap,
                        128 * NWP + s * MCH * D2,
                        [(NWP, 68), (D2, MCH), (1, NWP)],
                    ),
                    in_=_ap(
                        w2b[:, :],
                        s * MCH * NWP,
                        [(w2b[:, :].ap[0][0], 68), (NWP, MCH), (1, NWP)],
                    ),
                )

                # first w2t k-range for this sample right behind its w2d so
                # the first tap pair-group can start as soon as possible
                kh0, kn = 0, 14
                for g in range(G):
                    for h in range(NH):
                        nc.sync.dma_start(
                            out=_ap(
                                w2t_ap,
                                _p(s, g, h, 0) * P_W2T + kh0 * NWP,
                                [(P_W2T, MCH), (1, kn * NWP)],
                            ),
                            in_=_ap(
                                w2d_ap,
                                s * MCH * D2 + (g * KK + kh0) * NWP,
                                [(D2, MCH), (1, kn * NWP)],
                            ),
                        )

                # xu gathers on the SWDGE queue (16-engine, parallel with SP);
                # c-outer/m-inner so source reads walk DRAM monotonically
                for g in range(G):
                    for h in range(NH):
                        nc.gpsimd.dma_start(
                            out=_ap(
                                xu_ap,
                                _p(s, g, h, 0) * P_XU,
                                [(P_XU, MCH), (HR * W, C8), (1, 13 * W)],
                            ),
                            in_=_ap(
                                o1d_ap,
                                (g * GC + h * C8) * D1 + s * SPX,
                                [(NW, MCH), (D1, C8), (1, 13 * W)],
                            ),
                        )
            pfe_cm.__exit__(None, None, None)

            # remaining w2t k-ranges (consumed progressively by the taps)
            for kh0, kn in ((14, 14), (28, 14), (42, 7)):
                for s in range(S):
                    for g in range(G):
                        for h in range(NH):
                            nc.sync.dma_start(
                                out=_ap(
                                    w2t_ap,
                                    _p(s, g, h, 0) * P_W2T + kh0 * NWP,
                                    [(P_W2T, MCH), (1, kn * NWP)],
                                ),
                                in_=_ap(
                                    w2d_ap,
                                    s * MCH * D2 + (g * KK + kh0) * NWP,
                                    [(D2, MCH), (1, kn * NWP)],
                                ),
                            )

            # ---- halo expansion: XU -> XH / XH2, both on DVE (4x copies,
            # ~1.8us each; ACT would take 6.7us and sits on the critical
            # path into the first tap) ----
            for s in range(S):
                pl = slice(s * 64, (s + 1) * 64)
                nc.vector.tensor_copy(xh_v[pl, :, :, 3 : 3 + W], xu_v[pl])
                nc.vector.tensor_copy(xh2_v[pl, :, :, 4 : 4 + W], xu_v[pl])

            # ---- involution taps: kh-pairs merged into single DVE ops ----
            # Pair (kh, kh+1) for kh in {0,2,4} plus single kh=6, per kw.
            # Each pair op has a k2 dim (stride 62 in xh, 7*NWP in w2t,
            # ACCF in acc2) -> two partial sums in acc2's slots; one final
            # slot-add after all taps.
            # Channel split: channels 0..CP-1 of each (g,h) accumulate on the
            # PE (identity-stationary matmuls into PSUM, both tap slots into
            # one bank so no slot fold is needed); channels CP..7 accumulate
            # on DVE as before.  DVE multiplies for all 8 channels.
            ptap_cm = tc.tile_pool(name="psum_tap", bufs=1, space="PSUM")
            ptap = ptap_cm.__enter__()
            pacc = [
                ptap.tile(
                    [128, NWP], dt.float32, tag=f"pacc{c}", bufs=1, name=f"pacc{c}"
                )
                for c in range(CP)
            ]
            w2t_raw = w2t[:, :]
            first = True
            ti = 0
            nmm = [0] * CP
            for kh in (0, 2, 4, 6):
                npair = 2 if kh < 6 else 1
                for kw in (1, 3, 5, 0, 2, 4, 6):
                    if kw % 2 == 0:
                        srcap, psrc, base = xh[:, :], P_XH, kh * WP + kw
                    else:
                        srcap, psrc, base = xh2[:, :], P_XH2, kh * WP + kw + 1
                    k = kh * KS + kw
                    t = ti % 2
                    ti += 1
                    if first:
                        # first pair: DVE channels multiply straight into
                        # acc2; PE channels into tmp for the matmul chain
                        nc.vector.tensor_mul(
                            _ap(
                                acc2[:, :], CP * NWP,
                                [(P_A2, NP), (ACCF, npair), (NWP, CV), (1, RUN)],
                            ),
                            _ap(
                                srcap, base + CP * 868,
                                [(psrc, NP), (WP, npair), (868, CV), (1, RUN)],
                            ),
                            _ap(
                                w2t_raw, k * NWP,
                                [(P_W2T, NP), (KS * NWP, npair), (0, CV), (1, RUN)],
                            ),
                        )
                        nc.vector.tensor_mul(
                            _ap(
                                tmp2[t][:, :], 0,
                                [(P_T2[t], NP), (ACCF, npair), (NWP, CP), (1, RUN)],
                            ),
                            _ap(
                                srcap, base,
                                [(psrc, NP), (WP, npair), (868, CP), (1, RUN)],
                            ),
                            _ap(
                                w2t_raw, k * NWP,
                                [(P_W2T, NP), (KS * NWP, npair), (0, CP), (1, RUN)],
                            ),
                        )
                        first = False
                    else:
                        nc.vector.tensor_mul(
                            _ap(
                                tmp2[t][:, :], 0,
                                [(P_T2[t], NP), (ACCF, npair), (NWP, C8), (1, RUN)],
                            ),
                            _ap(
                                srcap, base,
                                [(psrc, NP), (WP, npair), (868, C8), (1, RUN)],
                            ),
                            _ap(
                                w2t_raw, k * NWP,
                                [(P_W2T, NP), (KS * NWP, npair), (0, C8), (1, RUN)],
                            ),
                        )
                        nc.vector.tensor_add(
                            _ap(
                                acc2[:, :], CP * NWP,
                                [(P_A2, NP), (ACCF, npair), (NWP, CV), (1, RUN)],
                            ),
                            _ap(
                                acc2[:, :], CP * NWP,
                                [(P_A2, NP), (ACCF, npair), (NWP, CV), (1, RUN)],
                            ),
                            _ap(
                                tmp2[t][:, :], CP * NWP,
                                [(P_T2[t], NP), (ACCF, npair), (NWP, CV), (1, RUN)],
                            ),
                        )
                    # PE accumulation for channels 0..CP-1 (both slots)
                    for sp in range(npair):
                        for c in range(CP):
                            nc.tensor.matmul(
                                pacc[c][:, :],
                                id_sb[:, :],
                                tmp2[t][:, sp * ACCF + c * NWP : sp * ACCF + (c + 1) * NWP],
                                start=(nmm[c] == 0),
                                stop=(nmm[c] == KK - 1),
                                skip_group_check=True,
                            )
                            nmm[c] += 1
            # fold slot1 into slot0 (DVE channels only)
            s0 = _ap(acc2[:, :], CP * NWP, [(P_A2, NP), (NWP, CV), (1, NWP)])
            s1 = _ap(acc2[:, :], ACCF + CP * NWP, [(P_A2, NP), (NWP, CV), (1, NWP)])
            nc.vector.tensor_add(s0, s0, s1)
            # evacuate the PE accumulators into acc2 slot0 (ACT, f32->bf16)
            for c in range(CP):
                nc.scalar.copy(acc2[:, c * NWP : (c + 1) * NWP], pacc[c][:, :])
            ptap_cm.__exit__(None, None, None)

            # ---- acc -> DRAM channel-major scatter (62-padded throughout;
            # the pad columns carry exact zeros / ignorable junk) ------------
            # per sample: scatter split across both queues, then that
            # sample's out2 readback immediately, so BN2(s0) can start while
            # sample 1 is still being scattered
            acd_ap = accd[:, :]
            D3 = acd_ap.ap[0][0]
            acc_ap = acc2[:, :]
            P_AC = P_A2
            SMW = MCH * NWP  # 3472 padded pixels per sample
            out2v = out1[:, 0 : S * SMW]
            for s in range(S):
                for g in range(G):
                    for h in range(NH):
                        eng = nc.gpsimd if (g * NH + h) % 2 == 0 else nc.sync
                        eng.dma_start(
                            out=_ap(
                                acd_ap,
                                (g * GC + h * C8) * D3 + s * MCH * NWP,
                                [(NWP, MCH), (D3, C8), (1, NWP)],
                            ),
                            in_=_ap(
                                acc_ap,
                                _p(s, g, h, 0) * P_AC,
                                [(P_AC, MCH), (NWP, C8), (1, NWP)],
                            ),
                        )
                # out2 reuses the (dead) out1 tile, 62-padded layout
                nc.sync.dma_start(
                    out=out2v[:, s * SMW : (s + 1) * SMW],
                    in_=accd[:, s * SMW : (s + 1) * SMW],
                )

            # ---- BN2 + ReLU; conv3 + BN3 + residual + ReLU ----------------
            pbe_cm = tc.tile_pool(name="psum_be", bufs=2, space="PSUM")
            pbe = pbe_cm.__enter__()
            w3t_v = w3t_sb[:, :].rearrange("p (k c) -> p k c", k=2)
            eng_flip = 0
            for s in range(S):
                obufs = [
                    opool.tile([128, HW], dt.bfloat16, tag="ob", name=f"ob{s}_{i}")
                    for i in range(2)
                ]
                for q in range(4):
                    rst = rpool.tile([CMID, 2 * NWP], dt.bfloat16, tag="rst")
                    nc.scalar.activation(
                        rst[:, :],
                        out2v[:, s * SMW + q * 2 * NWP : s * SMW + (q + 1) * 2 * NWP],
                        AF.Relu,
                        bias=b2v_sb[:, 0:1],
                        scale=s2v_sb[:, 0:1],
                    )
                    rst_v = rst[:, :].rearrange(
                        "p (m r w) -> p m r w", m=2, r=RH, w=WP
                    )
                    for oc in range(2):
                        # batch the two hf chunks per stationary (w3 then
                        # identity) so LDWEIGHTS loads 2x instead of 4x
                        pss = [
                            pbe.tile(
                                [128, NW], dt.float32, tag="ps", bufs=4,
                                name=f"ps3_{s}_{q}_{oc}_{hf}",
                            )
                            for hf in range(2)
                        ]
                        for hf in range(2):
                            nc.tensor.matmul(
                                pss[hf][:, :],
                                w3t_v[:, oc, :],
                                rst_v[:, hf, :, 0:W],
                                start=True,
                                stop=False,
                            )
                        for hf in range(2):
                            nx = (q * 2 + hf) * NW
                            nc.tensor.matmul(
                                pss[hf][:, :],
                                id_sb[:, :],
                                xbf_v[:, s, oc, nx : nx + NW],
                                start=False,
                                stop=True,
                            )
                        for hf in range(2):
                            nx = (q * 2 + hf) * NW
                            dst = obufs[oc][:, nx : nx + NW]
                            if eng_flip % 2 == 0:
                                nc.vector.tensor_scalar(
                                    dst,
                                    pss[hf][:, :],
                                    b3_sb[:, oc : oc + 1],
                                    0.0,
                                    op0=ALU.add,
                                    op1=ALU.max,
                                )
                            else:
                                nc.scalar.activation(
                                    dst, pss[hf][:, :], AF.Relu,
                                    bias=b3_sb[:, oc : oc + 1],
                                )
                            eng_flip += 1
                for oc in range(2):
                    nc.sync.dma_start(
                        out=out.ap()[s, oc * 128 : (oc + 1) * 128, :],
                        in_=obufs[oc][:, :],
                    )
            pbe_cm.__exit__(None, None, None)

    nc.compile()
    _CACHE["nc"] = nc
    return nc


def _f32(a):
    return np.ascontiguousarray(a, dtype=np.float32)


def prep_weights(inputs):
    """Host-side folding of BN scales into conv weights; bf16 casts."""
    f = inputs
    s1 = f["bn1_g"] / np.sqrt(f["bn1_v"] + EPS)
    b1_eff = f["bn1_b"] - f["bn1_m"] * s1
    w1t_eff = (_f32(f["conv1_w"]) * s1[:, None]).T          # [256, 64]

    si = f["inv_bn_g"] / np.sqrt(f["inv_bn_v"] + EPS)
    bi_eff = f["inv_bn_b"] - f["inv_bn_m"] * si
    c1t_eff = (_f32(f["inv_c1_w"]) * si[:, None]).T         # [64, 16]

    c2t_eff = _f32(f["inv_c2_w"]).T                         # [16, 196]
    b2c = _f32(f["inv_c2_b"])

    s2 = f["bn2_g"] / np.sqrt(f["bn2_v"] + EPS)
    b2n = f["bn2_b"] - f["bn2_m"] * s2

    s3 = f["bn3_g"] / np.sqrt(f["bn3_v"] + EPS)
    b3_eff = f["bn3_b"] - f["bn3_m"] * s3
    w3t_eff = (_f32(f["conv3_w"]) * s3[:, None]).T          # [64, 256]

    d = {}
    d["w1t"] = np.ascontiguousarray(w1t_eff.reshape(2, 128, CMID).astype(BF16))
    d["b1"] = _f32(b1_eff)[:, None]
    d["c1t"] = np.ascontiguousarray(c1t_eff.astype(BF16))
    d["bi"] = _f32(bi_eff)[:, None]
    d["c2t"] = np.ascontiguousarray(c2t_eff.astype(BF16))
    d["b2ca"] = _f32(b2c[0:128])[:, None]
    d["b2cb"] = _f32(b2c[128:196])[:, None]
    d["s2v"] = _f32(s2)[:, None]
    d["b2v"] = _f32(b2n)[:, None]
    d["w3t"] = np.ascontiguousarray(
        w3t_eff.reshape(CMID, 2, 128).transpose(1, 0, 2).astype(BF16)
    )
    d["b3"] = _f32(b3_eff.reshape(2, 128).T)
    d["ident"] = np.ascontiguousarray(np.eye(128, dtype=np.float32).astype(BF16))
    return d


def make_in_maps(inputs):
    prep = prep_weights(inputs)
    x = _f32(inputs["x"]).reshape(16, CIN, HW)
    in_maps = []
    for i in range(N_CORES):
        m = dict(prep)
        m["xin"] = np.ascontiguousarray(x[S * i : S * i + S])
        in_maps.append(m)
    return in_maps


def kernel(**inputs):
    from concourse.bass_utils import run_bass_kernel_spmd

    nc = build_module()
    in_maps = make_in_maps(inputs)
    res = run_bass_kernel_spmd(nc, in_maps, core_ids=list(range(N_CORES)))
    outs = [
        np.asarray(res.results[i]["out"], dtype=np.float32).reshape(S, CIN, H, W)
        for i in range(N_CORES)
    ]
    return np.concatenate(outs, axis=0).astype(np.float32)



# revision 26
# speedup vs baseline: 1.0147x; 1.0147x over previous
"""Trainium2 Bass kernel for nn_Bottleneck_5669356834470 (ResNet bottleneck
with an involution middle layer) — v4.

Sharding: data-parallel over batch. 16 samples / 8 cores = 2 samples/core.

Key changes vs v3:
  * tap accumulation split across engines: channels 0-3 of each (group,
    half) accumulate on the (otherwise idle) PE via identity-stationary
    matmuls into 4 PSUM banks; channels 4-7 keep the DVE add path.  DVE
    only multiplies for the PE channels, cutting its tap cost ~25%.
  * inv_c1 PSUM evac moved DVE -> ACT (PSUM-input DVE ops run 1x).
  * conv1 / conv3 matmuls batched per stationary operand (2-chunk groups)
    to stop LDWEIGHTS thrash; PSUM bufs rebalanced 2+2+4 (ps/psw/tap-acc).
  * x load split into half-sample DMAs; w2 DRAM staging issued per sample
    so the first tap range lands sooner.
"""

import sys

sys.path.insert(0, "/opt/trn_rl_repo")

import numpy as np
import ml_dtypes

BF16 = ml_dtypes.bfloat16

S = 2            # samples per core
N_CORES = 8
CIN = 256
CMID = 64
G = 4            # involution groups
GC = 16          # channels per group
C8 = 8           # channels per (group, half)
NH = 2           # channel halves per group
KS = 7           # involution kernel size
KK = KS * KS     # 49
R = 16           # dyn-weight bottleneck channels
H = W = 56
HW = H * W       # 3136
RH = 7           # output rows per partition chunk
MCH = 8          # row chunks per (s, g, h)
NP = 128         # partitions = S*G*NH*MCH
NW = RH * W      # 392: matmul / staging chunk (conv1 side)
NWP = RH * 62    # 434: row-padded pixel chunk (w2 / tap side)
NCH = 8          # spatial chunks per sample
HR = 14          # halo rows stored per chunk (13 valid + 1 zero pad)
WP = 62          # padded row width
RUN = 6 * WP + W     # 428: contiguous tap run (7 rows incl inter-row pads)
PAD = 3 * W      # 168: zero margin per sample in out1d
SPX = PAD + HW + PAD   # 3472: out1d pixels per sample
XUF = C8 * HR * W    # 6272 free elems per XU partition
XHF = C8 * HR * WP   # 6944 free elems per XH partition
W2F = KK * NWP       # 21266 free elems per W2T partition (row-padded)
ACCF = C8 * NWP      # 3472 acc free elems per partition (row-padded)
ACCC = C8 * NW       # 3136 compact acc free elems per partition
EPS = 1e-5
CP = 7           # channels per (g,h) accumulated on the PE (PSUM banks)
CV = C8 - CP     # channels per (g,h) accumulated on DVE

# Tap multiplies all on DVE (GpSimd tensor ops contend for SBUF and degrade
# DVE throughput ~4x, measured on HW).  Taps ordered by k so they can start
# as soon as the first k-range of the weight gather lands.

_CACHE = {}


def _p(s, g, h, m):
    return ((s * G + g) * NH + h) * MCH + m


def _ap(tile_ap, off, dims):
    """Raw strided AP on a tile's underlying tensor. dims=[(step,count),...]
    in elements; for SBUF the partition stride is ap[0][0] of the base AP."""
    import bass_rust

    return bass_rust.AP(tile_ap.tensor, tile_ap.offset + off, [list(d) for d in dims])


def build_module():
    if "nc" in _CACHE:
        return _CACHE["nc"]
    import concourse.bacc as bacc
    import concourse.mybir as mybir
    import concourse.tile as tile

    dt = mybir.dt
    AF = mybir.ActivationFunctionType
    ALU = mybir.AluOpType

    nc = bacc.Bacc("TRN2", debug=False, num_devices=N_CORES)

    # ---- DRAM I/O ----------------------------------------------------------
    xin = nc.dram_tensor("xin", [S, CIN, HW], dt.float32, kind="ExternalInput")
    w1t = nc.dram_tensor("w1t", [2, 128, CMID], dt.bfloat16, kind="ExternalInput")
    b1 = nc.dram_tensor("b1", [CMID, 1], dt.float32, kind="ExternalInput")
    c1t = nc.dram_tensor("c1t", [CMID, R], dt.bfloat16, kind="ExternalInput")
    bi = nc.dram_tensor("bi", [R, 1], dt.float32, kind="ExternalInput")
    c2t = nc.dram_tensor("c2t", [R, G * KK], dt.bfloat16, kind="ExternalInput")
    b2ca = nc.dram_tensor("b2ca", [128, 1], dt.float32, kind="ExternalInput")
    b2cb = nc.dram_tensor("b2cb", [68, 1], dt.float32, kind="ExternalInput")
    s2v = nc.dram_tensor("s2v", [CMID, 1], dt.float32, kind="ExternalInput")
    b2v = nc.dram_tensor("b2v", [CMID, 1], dt.float32, kind="ExternalInput")
    w3t = nc.dram_tensor("w3t", [2, CMID, 128], dt.bfloat16, kind="ExternalInput")
    b3 = nc.dram_tensor("b3", [128, 2], dt.float32, kind="ExternalInput")
    ident = nc.dram_tensor("ident", [128, 128], dt.bfloat16, kind="ExternalInput")
    out = nc.dram_tensor("out", [S, CIN, HW], dt.bfloat16, kind="ExternalOutput")

    with tile.TileContext(nc) as tc:
        with (
            tc.tile_pool(name="consts", bufs=1) as cpool,
            tc.tile_pool(name="big", bufs=1) as bpool,
            tc.tile_pool(name="zst", bufs=4) as zpool,
            tc.tile_pool(name="rst", bufs=2) as rpool,
            tc.tile_pool(name="ob", bufs=2) as opool,
            tc.tile_pool(name="dstage", bufs=1, space="DRAM") as dpool,
        ):
            # ---- constants -> SBUF ----------------------------------------
            w1t_sb = cpool.tile([128, 2 * CMID], dt.bfloat16, tag="w1t")
            nc.sync.dma_start(
                out=w1t_sb[:, :].rearrange("p (k c) -> p k c", k=2),
                in_=w1t.ap().rearrange("k p c -> p k c"),
            )
            b1_sb = cpool.tile([CMID, 1], dt.float32, tag="b1")
            nc.sync.dma_start(out=b1_sb[:, :], in_=b1.ap())
            c1t_sb = cpool.tile([CMID, R], dt.bfloat16, tag="c1t")
            nc.sync.dma_start(out=c1t_sb[:, :], in_=c1t.ap())
            bi_sb = cpool.tile([R, 1], dt.float32, tag="bi")
            nc.sync.dma_start(out=bi_sb[:, :], in_=bi.ap())
            c2t_sb = cpool.tile([R, G * KK], dt.bfloat16, tag="c2t")
            nc.sync.dma_start(out=c2t_sb[:, :], in_=c2t.ap())
            b2ca_sb = cpool.tile([128, 1], dt.float32, tag="b2ca")
            nc.sync.dma_start(out=b2ca_sb[:, :], in_=b2ca.ap())
            b2cb_sb = cpool.tile([68, 1], dt.float32, tag="b2cb")
            nc.sync.dma_start(out=b2cb_sb[:, :], in_=b2cb.ap())
            s2v_sb = cpool.tile([CMID, 1], dt.float32, tag="s2v")
            nc.sync.dma_start(out=s2v_sb[:, :], in_=s2v.ap())
            b2v_sb = cpool.tile([CMID, 1], dt.float32, tag="b2v")
            nc.sync.dma_start(out=b2v_sb[:, :], in_=b2v.ap())
            w3t_sb = cpool.tile([CMID, 2 * 128], dt.bfloat16, tag="w3t")
            nc.sync.dma_start(
                out=w3t_sb[:, :].rearrange("p (k c) -> p k c", k=2),
                in_=w3t.ap().rearrange("k p c -> p k c"),
            )
            b3_sb = cpool.tile([128, 2], dt.float32, tag="b3")
            nc.sync.dma_start(out=b3_sb[:, :], in_=b3.ap())
            id_sb = cpool.tile([128, 128], dt.bfloat16, tag="ident")
            nc.sync.dma_start(out=id_sb[:, :], in_=ident.ap())

            # ---- big SBUF tiles -------------------------------------------
            xbf = bpool.tile([128, S * 2 * HW], dt.bfloat16, tag="xbf")
            # sized for its out2 reuse (62-padded layout needs S*MCH*NWP=6944)
            out1 = bpool.tile([CMID, S * MCH * NWP], dt.bfloat16, tag="out1")
            w2a = bpool.tile([128, S * MCH * NWP], dt.bfloat16, tag="w2a")
            w2b = bpool.tile([68, S * MCH * NWP], dt.bfloat16, tag="w2b")
            xu = bpool.tile([NP, XUF], dt.bfloat16, tag="xu")
            xh = bpool.tile([NP, XHF], dt.bfloat16, tag="xh")
            xh2 = bpool.tile([NP, XHF], dt.bfloat16, tag="xh2")
            w2t = bpool.tile([NP, W2F], dt.bfloat16, tag="w2t")
            acc2 = bpool.tile([NP, 2 * ACCF], dt.bfloat16, tag="acc2")
            tmp2 = [
                bpool.tile([NP, 2 * ACCF], dt.bfloat16, tag=f"tmp2_{i}", name=f"tmp2_{i}")
                for i in range(2)
            ]
            out2 = out1  # out1 is dead after the xu gathers; reuse for out2
            zt = bpool.tile([CMID, PAD], dt.bfloat16, tag="zt")
            zst2 = [
                bpool.tile([R, NWP], dt.bfloat16, tag=f"zst{i}", name=f"zst{i}")
                for i in range(2)
            ]

            xbf_v = xbf[:, :].rearrange("p (s k f) -> p s k f", s=S, k=2)
            xu_v = xu[:, :].rearrange("p (c r w) -> p c r w", c=C8, r=HR, w=W)
            xh_v = xh[:, :].rearrange("p (c r w) -> p c r w", c=C8, r=HR, w=WP)
            xh2_v = xh2[:, :].rearrange("p (c r w) -> p c r w", c=C8, r=HR, w=WP)
            # flat views for the long-run tap ops
            xh_f = xh[:, :].rearrange("p (c f) -> p c f", c=C8)
            xh2_f = xh2[:, :].rearrange("p (c f) -> p c f", c=C8)
            P_XH = xh[:, :].ap[0][0]
            P_XH2 = xh2[:, :].ap[0][0]
            P_A2 = acc2[:, :].ap[0][0]
            P_T2 = [t[:, :].ap[0][0] for t in tmp2]

            # ---- DRAM staging ---------------------------------------------
            # w2d is (s,m)-major so the w2t gather per partition is one
            # contiguous multi-KB run (DRAM-sequential, not 111KB strides).
            out1d = dpool.tile([CMID, S * SPX], dt.bfloat16, tag="out1d")
            w2d = dpool.tile([S * MCH, G * KK * NWP], dt.bfloat16, tag="w2d")
            accd = dpool.tile([CMID, S * MCH * NWP], dt.bfloat16, tag="accd")

            # ---- memzeros (pads for halo tensors) -------------------------
            # all on DVE (4x memset) so the ACT queue is free for the conv1
            # evacuations from the very start
            nc.vector.memset(xu[:, :], 0.0)
            nc.vector.memset(xh[:, :], 0.0)
            nc.vector.memset(xh2[:, :], 0.0)
            nc.vector.memset(zt[:, :], 0.0)
            nc.vector.memset(acc2[:, :], 0.0)
            for z in zst2:
                nc.vector.memset(z[:, :], 0.0)

            # zero margins of out1d so halo gathers read zeros off the edges
            for s in range(S):
                nc.sync.dma_start(
                    out=out1d[:, s * SPX : s * SPX + PAD], in_=zt[:, :]
                )
                nc.sync.dma_start(
                    out=out1d[:, s * SPX + PAD + HW : (s + 1) * SPX], in_=zt[:, :]
                )

            # ---- x load (f32 -> bf16 cast; SWDGE on gpsimd) ---------------
            # split in half-sample pieces so conv1 can start on the first
            # chunks while the rest streams in
            HWH = HW // 2
            for s in range(S):
                for hh in range(2):
                    for kc in range(2):
                        nc.gpsimd.dma_start(
                            out=xbf_v[:, s, kc, hh * HWH : (hh + 1) * HWH],
                            in_=xin.ap()[
                                s, kc * 128 : (kc + 1) * 128, hh * HWH : (hh + 1) * HWH
                            ],
                        )

            # ---- per-sample front end: conv1 / inv_c1 / inv_c2 ------------
            w1t_v = w1t_sb[:, :].rearrange("p (k c) -> p k c", k=2)
            o1d_ap = out1d[:, :]
            w2d_ap = w2d[:, :]
            xu_ap = xu[:, :]
            w2t_ap = w2t[:, :]
            D1 = o1d_ap.ap[0][0]
            D2 = w2d_ap.ap[0][0]
            P_XU = xu_ap.ap[0][0]
            P_W2T = w2t_ap.ap[0][0]

            pfe_cm = tc.tile_pool(name="psum_fe", bufs=2, space="PSUM")
            pfe = pfe_cm.__enter__()
            for s in range(S):
                for n0 in range(0, NCH, 2):
                    # conv1 (256->64) + BN1 + ReLU   [ACT evac]
                    # 2-chunk batches per stationary operand: LDWEIGHTS for
                    # w1t[kc] loads once per pair instead of per chunk.
                    pss = []
                    for kc in range(2):
                        for dn in range(2):
                            n = n0 + dn
                            if kc == 0:
                                pss.append(
                                    pfe.tile(
                                        [128, NW], dt.float32, tag="ps", bufs=3,
                                        name=f"ps_{s}_{n}",
                                    )
                                )
                            nc.tensor.matmul(
                                pss[dn][:CMID, :],
                                w1t_v[:, kc, :],
                                xbf_v[:, s, kc, n * NW : (n + 1) * NW],
                                start=(kc == 0),
                                stop=(kc == 1),
                            )
                    for dn in range(2):
                        n = n0 + dn
                        sl = slice(s * HW + n * NW, s * HW + (n + 1) * NW)
                        nc.scalar.activation(
                            out1[:, sl], pss[dn][:CMID, :], AF.Relu, bias=b1_sb[:, 0:1]
                        )
                    for dn in range(2):
                        n = n0 + dn
                        sl = slice(s * HW + n * NW, s * HW + (n + 1) * NW)
                        # inv_c1 (64->16) + BN + ReLU    [ACT evac]
                        # zst is row-padded to 62-wide rows; the pad columns
                        # keep stale (finite) values which flow through inv_c2
                        # into pad weight columns that only multiply zeros.
                        ps1 = pfe.tile([128, NW], dt.float32, tag="ps", bufs=3)
                        nc.tensor.matmul(
                            ps1[:R, :], c1t_sb[:, :], out1[:, sl], start=True, stop=True
                        )
                        zst = zst2[n % 2]
                        zst_v = zst[:, :].rearrange("p (r w) -> p r w", r=RH, w=WP)
                        nc.scalar.activation(
                            zst_v[:, :, 0:W],
                            ps1[:R, :].rearrange("p (r w) -> p r w", r=RH, w=W),
                            AF.Relu,
                            bias=bi_sb[:, 0:1],
                        )
                        # inv_c2 (16->196) + bias        [DVE + ACT evacs]
                        psa = pfe.tile([128, NWP], dt.float32, tag="psw", bufs=4)
                        psb = pfe.tile([128, NWP], dt.float32, tag="psw", bufs=4)
                        nc.tensor.matmul(
                            psa[:, :], c2t_sb[:, 0:128], zst[:, :], start=True, stop=True
                        )
                        nc.tensor.matmul(
                            psb[:68, :], c2t_sb[:, 128:196], zst[:, :],
                            start=True, stop=True,
                        )
                        nsl = slice((s * MCH + n) * NWP, (s * MCH + n + 1) * NWP)
                        nc.vector.tensor_scalar(
                            w2a[:, nsl], psa[:, :], b2ca_sb[:, 0:1], None, op0=ALU.add
                        )
                        nc.scalar.activation(
                            w2b[:, nsl], psb[:68, :], AF.Identity, bias=b2cb_sb[:, 0:1]
                        )

                    # stage out1 through DRAM as each 2-chunk batch finishes
                    nc.sync.dma_start(
                        out=out1d[
                            :,
                            s * SPX + PAD + n0 * NW : s * SPX + PAD + (n0 + 2) * NW,
                        ],
                        in_=out1[:, s * HW + n0 * NW : s * HW + (n0 + 2) * NW],
                    )

                # w2 -> (s,m)-major DRAM per sample (dst runs gk-seq 868B)
                nc.sync.dma_start(
                    out=_ap(w2d_ap, s * MCH * D2, [(NWP, 128), (D2, MCH), (1, NWP)]),
                    in_=_ap(
                        w2a[:, :],
                        s * MCH * NWP,
                        [(w2a[:, :].ap[0][0], 128), (NWP, MCH), (1, NWP)],
                    ),
                )
                nc.sync.dma_start(
                    out=_ap(
                        w2d_ap,
                        128 * NWP + s * MCH * D2,
                        [(NWP, 68), (D2, MCH), (1, NWP)],
                    ),
                    in_=_ap(
                        w2b[:, :],
                        s * MCH * NWP,
                        [(w2b[:, :].ap[0][0], 68), (NWP, MCH), (1, NWP)],
                    ),
                )

                # first w2t k-range for this sample right behind its w2d so
                # the first tap pair-group can start as soon as possible
                kh0, kn = 0, 14
                for g in range(G):
                    for h in range(NH):
                        nc.sync.dma_start(
                            out=_ap(
                                w2t_ap,
                                _p(s, g, h, 0) * P_W2T + kh0 * NWP,
                                [(P_W2T, MCH), (1, kn * NWP)],
                            ),
                            in_=_ap(
                                w2d_ap,
                                s * MCH * D2 + (g * KK + kh0) * NWP,
                                [(D2, MCH), (1, kn * NWP)],
                            ),
                        )

                # xu gathers on the SWDGE queue (16-engine, parallel with SP);
                # c-outer/m-inner so source reads walk DRAM monotonically
                for g in range(G):
                    for h in range(NH):
                        nc.gpsimd.dma_start(
                            out=_ap(
                                xu_ap,
                                _p(s, g, h, 0) * P_XU,
                                [(P_XU, MCH), (HR * W, C8), (1, 13 * W)],
                            ),
                            in_=_ap(
                                o1d_ap,
                                (g * GC + h * C8) * D1 + s * SPX,
                                [(NW, MCH), (D1, C8), (1, 13 * W)],
                            ),
                        )
            pfe_cm.__exit__(None, None, None)

            # remaining w2t k-ranges (consumed progressively by the taps).
            # On the SWDGE queue BEHIND the xu gathers: their 4MB would
            # otherwise saturate the DMA engines exactly when the xu(s1) ->
            # xh chain (the critical path into the first tap) needs them.
            for kh0, kn in ((14, 14), (28, 14), (42, 7)):
                for s in range(S):
                    for g in range(G):
                        for h in range(NH):
                            nc.gpsimd.dma_start(
                                out=_ap(
                                    w2t_ap,
                                    _p(s, g, h, 0) * P_W2T + kh0 * NWP,
                                    [(P_W2T, MCH), (1, kn * NWP)],
                                ),
                                in_=_ap(
                                    w2d_ap,
                                    s * MCH * D2 + (g * KK + kh0) * NWP,
                                    [(D2, MCH), (1, kn * NWP)],
                                ),
                            )

            # ---- halo expansion: XU -> XH / XH2, both on DVE (4x copies,
            # ~1.8us each; ACT would take 6.7us and sits on the critical
            # path into the first tap) ----
            for s in range(S):
                pl = slice(s * 64, (s + 1) * 64)
                nc.vector.tensor_copy(xh_v[pl, :, :, 3 : 3 + W], xu_v[pl])
                nc.vector.tensor_copy(xh2_v[pl, :, :, 4 : 4 + W], xu_v[pl])

            # ---- involution taps: kh-pairs merged into single DVE ops ----
            # Pair (kh, kh+1) for kh in {0,2,4} plus single kh=6, per kw.
            # Each pair op has a k2 dim (stride 62 in xh, 7*NWP in w2t,
            # ACCF in acc2) -> two partial sums in acc2's slots; one final
            # slot-add after all taps.
            # Channel split: channels 0..CP-1 of each (g,h) accumulate on the
            # PE (identity-stationary matmuls into PSUM, both tap slots into
            # one bank so no slot fold is needed); channels CP..7 accumulate
            # on DVE as before.  DVE multiplies for all 8 channels.
            ptap_cm = tc.tile_pool(name="psum_tap", bufs=1, space="PSUM")
            ptap = ptap_cm.__enter__()
            pacc = [
                ptap.tile(
                    [128, NWP], dt.float32, tag=f"pacc{c}", bufs=1, name=f"pacc{c}"
                )
                for c in range(CP)
            ]
            w2t_raw = w2t[:, :]
            first = True
            ti = 0
            nmm = [0] * CP
            for kh in (0, 2, 4, 6):
                npair = 2 if kh < 6 else 1
                for kw in (1, 3, 5, 0, 2, 4, 6):
                    if kw % 2 == 0:
                        srcap, psrc, base = xh[:, :], P_XH, kh * WP + kw
                    else:
                        srcap, psrc, base = xh2[:, :], P_XH2, kh * WP + kw + 1
                    k = kh * KS + kw
                    t = ti % 2
                    ti += 1
                    if first:
                        # first pair: DVE channels multiply straight into
                        # acc2; PE channels into tmp for the matmul chain
                        nc.vector.tensor_mul(
                            _ap(
                                acc2[:, :], CP * NWP,
                                [(P_A2, NP), (ACCF, npair), (NWP, CV), (1, RUN)],
                            ),
                            _ap(
                                srcap, base + CP * 868,
                                [(psrc, NP), (WP, npair), (868, CV), (1, RUN)],
                            ),
                            _ap(
                                w2t_raw, k * NWP,
                                [(P_W2T, NP), (KS * NWP, npair), (0, CV), (1, RUN)],
                            ),
                        )
                        nc.vector.tensor_mul(
                            _ap(
                                tmp2[t][:, :], 0,
                                [(P_T2[t], NP), (ACCF, npair), (NWP, CP), (1, RUN)],
                            ),
                            _ap(
                                srcap, base,
                                [(psrc, NP), (WP, npair), (868, CP), (1, RUN)],
                            ),
                            _ap(
                                w2t_raw, k * NWP,
                                [(P_W2T, NP), (KS * NWP, npair), (0, CP), (1, RUN)],
                            ),
                        )
                        first = False
                    else:
                        nc.vector.tensor_mul(
                            _ap(
                                tmp2[t][:, :], 0,
                                [(P_T2[t], NP), (ACCF, npair), (NWP, C8), (1, RUN)],
                            ),
                            _ap(
                                srcap, base,
                                [(psrc, NP), (WP, npair), (868, C8), (1, RUN)],
                            ),
                            _ap(
                                w2t_raw, k * NWP,
                                [(P_W2T, NP), (KS * NWP, npair), (0, C8), (1, RUN)],
                            ),
                        )
                        nc.vector.tensor_add(
                            _ap(
                                acc2[:, :], CP * NWP,
                                [(P_A2, NP), (ACCF, npair), (NWP, CV), (1, RUN)],
                            ),
                            _ap(
                                acc2[:, :], CP * NWP,
                                [(P_A2, NP), (ACCF, npair), (NWP, CV), (1, RUN)],
                            ),
                            _ap(
                                tmp2[t][:, :], CP * NWP,
                                [(P_T2[t], NP), (ACCF, npair), (NWP, CV), (1, RUN)],
                            ),
                        )
                    # PE accumulation for channels 0..CP-1 (both slots)
                    for sp in range(npair):
                        for c in range(CP):
                            nc.tensor.matmul(
                                pacc[c][:, :],
                                id_sb[:, :],
                                tmp2[t][:, sp * ACCF + c * NWP : sp * ACCF + (c + 1) * NWP],
                                start=(nmm[c] == 0),
                                stop=(nmm[c] == KK - 1),
                                skip_group_check=True,
                            )
                            nmm[c] += 1
            # fold slot1 into slot0 (DVE channels only)
            s0 = _ap(acc2[:, :], CP * NWP, [(P_A2, NP), (NWP, CV), (1, NWP)])
            s1 = _ap(acc2[:, :], ACCF + CP * NWP, [(P_A2, NP), (NWP, CV), (1, NWP)])
            nc.vector.tensor_add(s0, s0, s1)
            # evacuate the PE accumulators into acc2 slot0 (split ACT/DVE)
            for c in range(CP):
                if c % 2 == 0:
                    nc.scalar.copy(acc2[:, c * NWP : (c + 1) * NWP], pacc[c][:, :])
                else:
                    nc.vector.tensor_copy(
                        acc2[:, c * NWP : (c + 1) * NWP], pacc[c][:, :]
                    )
            ptap_cm.__exit__(None, None, None)

            # ---- acc -> DRAM channel-major scatter (62-padded throughout;
            # the pad columns carry exact zeros / ignorable junk) ------------
            # per sample: scatter split across both queues, then that
            # sample's out2 readback immediately, so BN2(s0) can start while
            # sample 1 is still being scattered
            acd_ap = accd[:, :]
            D3 = acd_ap.ap[0][0]
            acc_ap = acc2[:, :]
            P_AC = P_A2
            SMW = MCH * NWP  # 3472 padded pixels per sample
            out2v = out1[:, 0 : S * SMW]
            for s in range(S):
                for g in range(G):
                    for h in range(NH):
                        eng = nc.sync if s == 0 else nc.gpsimd
                        eng.dma_start(
                            out=_ap(
                                acd_ap,
                                (g * GC + h * C8) * D3 + s * MCH * NWP,
                                [(NWP, MCH), (D3, C8), (1, NWP)],
                            ),
                            in_=_ap(
                                acc_ap,
                                _p(s, g, h, 0) * P_AC,
                                [(P_AC, MCH), (NWP, C8), (1, NWP)],
                            ),
                        )
                # out2 reuses the (dead) out1 tile, 62-padded layout
                nc.sync.dma_start(
                    out=out2v[:, s * SMW : (s + 1) * SMW],
                    in_=accd[:, s * SMW : (s + 1) * SMW],
                )

            # ---- BN2 + ReLU; conv3 + BN3 + residual + ReLU ----------------
            pbe_cm = tc.tile_pool(name="psum_be", bufs=2, space="PSUM")
            pbe = pbe_cm.__enter__()
            w3t_v = w3t_sb[:, :].rearrange("p (k c) -> p k c", k=2)
            eng_flip = 0
            for s in range(S):
                obufs = [
                    opool.tile([128, HW], dt.bfloat16, tag="ob", name=f"ob{s}_{i}")
                    for i in range(2)
                ]
                for q in range(4):
                    rst = rpool.tile([CMID, 2 * NWP], dt.bfloat16, tag="rst")
                    nc.scalar.activation(
                        rst[:, :],
                        out2v[:, s * SMW + q * 2 * NWP : s * SMW + (q + 1) * 2 * NWP],
                        AF.Relu,
                        bias=b2v_sb[:, 0:1],
                        scale=s2v_sb[:, 0:1],
                    )
                    rst_v = rst[:, :].rearrange(
                        "p (m r w) -> p m r w", m=2, r=RH, w=WP
                    )
                    for oc in range(2):
                        # batch the two hf chunks per stationary (w3 then
                        # identity) so LDWEIGHTS loads 2x instead of 4x
                        pss = [
                            pbe.tile(
                                [128, NW], dt.float32, tag="ps", bufs=4,
                                name=f"ps3_{s}_{q}_{oc}_{hf}",
                            )
                            for hf in range(2)
                        ]
                        for hf in range(2):
                            nc.tensor.matmul(
                                pss[hf][:, :],
                                w3t_v[:, oc, :],
                                rst_v[:, hf, :, 0:W],
                                start=True,
                                stop=False,
                            )
                        for hf in range(2):
                            nx = (q * 2 + hf) * NW
                            nc.tensor.matmul(
                                pss[hf][:, :],
                                id_sb[:, :],
                                xbf_v[:, s, oc, nx : nx + NW],
                                start=False,
                                stop=True,
                            )
                        for hf in range(2):
                            nx = (q * 2 + hf) * NW
                            dst = obufs[oc][:, nx : nx + NW]
                            if eng_flip % 2 == 0:
                                nc.vector.tensor_scalar(
                                    dst,
                                    pss[hf][:, :],
                                    b3_sb[:, oc : oc + 1],
                                    0.0,
                                    op0=ALU.add,
                                    op1=ALU.max,
                                )
                            else:
                                nc.scalar.activation(
                                    dst, pss[hf][:, :], AF.Relu,
                                    bias=b3_sb[:, oc : oc + 1],
                                )
                            eng_flip += 1
                for oc in range(2):
                    nc.sync.dma_start(
                        out=out.ap()[s, oc * 128 : (oc + 1) * 128, :],
                        in_=obufs[oc][:, :],
                    )
            pbe_cm.__exit__(None, None, None)

    nc.compile()
    _CACHE["nc"] = nc
    return nc


def _f32(a):
    return np.ascontiguousarray(a, dtype=np.float32)


def prep_weights(inputs):
    """Host-side folding of BN scales into conv weights; bf16 casts."""
    f = inputs
    s1 = f["bn1_g"] / np.sqrt(f["bn1_v"] + EPS)
    b1_eff = f["bn1_b"] - f["bn1_m"] * s1
    w1t_eff = (_f32(f["conv1_w"]) * s1[:, None]).T          # [256, 64]

    si = f["inv_bn_g"] / np.sqrt(f["inv_bn_v"] + EPS)
    bi_eff = f["inv_bn_b"] - f["inv_bn_m"] * si
    c1t_eff = (_f32(f["inv_c1_w"]) * si[:, None]).T         # [64, 16]

    c2t_eff = _f32(f["inv_c2_w"]).T                         # [16, 196]
    b2c = _f32(f["inv_c2_b"])

    s2 = f["bn2_g"] / np.sqrt(f["bn2_v"] + EPS)
    b2n = f["bn2_b"] - f["bn2_m"] * s2

    s3 = f["bn3_g"] / np.sqrt(f["bn3_v"] + EPS)
    b3_eff = f["bn3_b"] - f["bn3_m"] * s3
    w3t_eff = (_f32(f["conv3_w"]) * s3[:, None]).T          # [64, 256]

    d = {}
    d["w1t"] = np.ascontiguousarray(w1t_eff.reshape(2, 128, CMID).astype(BF16))
    d["b1"] = _f32(b1_eff)[:, None]
    d["c1t"] = np.ascontiguousarray(c1t_eff.astype(BF16))
    d["bi"] = _f32(bi_eff)[:, None]
    d["c2t"] = np.ascontiguousarray(c2t_eff.astype(BF16))
    d["b2ca"] = _f32(b2c[0:128])[:, None]
    d["b2cb"] = _f32(b2c[128:196])[:, None]
    d["s2v"] = _f32(s2)[:, None]
    d["b2v"] = _f32(b2n)[:, None]
    d["w3t"] = np.ascontiguousarray(
        w3t_eff.reshape(CMID, 2, 128).transpose(1, 0, 2).astype(BF16)
    )
    d["b3"] = _f32(b3_eff.reshape(2, 128).T)
    d["ident"] = np.ascontiguousarray(np.eye(128, dtype=np.float32).astype(BF16))
    return d


def make_in_maps(inputs):
    prep = prep_weights(inputs)
    x = _f32(inputs["x"]).reshape(16, CIN, HW)
    in_maps = []
    for i in range(N_CORES):
        m = dict(prep)
        m["xin"] = np.ascontiguousarray(x[S * i : S * i + S])
        in_maps.append(m)
    return in_maps


def kernel(**inputs):
    from concourse.bass_utils import run_bass_kernel_spmd

    nc = build_module()
    in_maps = make_in_maps(inputs)
    res = run_bass_kernel_spmd(nc, in_maps, core_ids=list(range(N_CORES)))
    outs = [
        np.asarray(res.results[i]["out"], dtype=np.float32).reshape(S, CIN, H, W)
        for i in range(N_CORES)
    ]
    return np.concatenate(outs, axis=0).astype(np.float32)



# revision 30
# speedup vs baseline: 1.0662x; 1.0508x over previous
"""Trainium2 Bass kernel for nn_Bottleneck_5669356834470 (ResNet bottleneck
with an involution middle layer) — v4.

Sharding: data-parallel over batch. 16 samples / 8 cores = 2 samples/core.

Key changes vs v3:
  * tap accumulation split across engines: channels 0-3 of each (group,
    half) accumulate on the (otherwise idle) PE via identity-stationary
    matmuls into 4 PSUM banks; channels 4-7 keep the DVE add path.  DVE
    only multiplies for the PE channels, cutting its tap cost ~25%.
  * inv_c1 PSUM evac moved DVE -> ACT (PSUM-input DVE ops run 1x).
  * conv1 / conv3 matmuls batched per stationary operand (2-chunk groups)
    to stop LDWEIGHTS thrash; PSUM bufs rebalanced 2+2+4 (ps/psw/tap-acc).
  * x load split into half-sample DMAs; w2 DRAM staging issued per sample
    so the first tap range lands sooner.
"""

import sys

sys.path.insert(0, "/opt/trn_rl_repo")

import numpy as np
import ml_dtypes

BF16 = ml_dtypes.bfloat16

S = 2            # samples per core
N_CORES = 8
CIN = 256
CMID = 64
G = 4            # involution groups
GC = 16          # channels per group
C8 = 8           # channels per (group, half)
NH = 2           # channel halves per group
KS = 7           # involution kernel size
KK = KS * KS     # 49
R = 16           # dyn-weight bottleneck channels
H = W = 56
HW = H * W       # 3136
RH = 7           # output rows per partition chunk
MCH = 8          # row chunks per (s, g, h)
NP = 128         # partitions = S*G*NH*MCH
NW = RH * W      # 392: matmul / staging chunk (conv1 side)
NWP = RH * 62    # 434: row-padded pixel chunk (w2 / tap side)
NCH = 8          # spatial chunks per sample
HR = 14          # halo rows stored per chunk (13 valid + 1 zero pad)
WP = 62          # padded row width
RUN = 6 * WP + W     # 428: contiguous tap run (7 rows incl inter-row pads)
PAD = 3 * W      # 168: zero margin per sample in out1d
SPX = PAD + HW + PAD   # 3472: out1d pixels per sample
XUF = C8 * HR * W    # 6272 free elems per XU partition
XHF = C8 * HR * WP   # 6944 free elems per XH partition
W2F = KK * NWP       # 21266 free elems per W2T partition (row-padded)
ACCF = C8 * NWP      # 3472 acc free elems per partition (row-padded)
ACCC = C8 * NW       # 3136 compact acc free elems per partition
EPS = 1e-5
CP = 7           # channels per (g,h) accumulated on the PE (PSUM banks)
CV = C8 - CP     # channels per (g,h) accumulated on DVE

# Tap multiplies all on DVE (GpSimd tensor ops contend for SBUF and degrade
# DVE throughput ~4x, measured on HW).  Taps ordered by k so they can start
# as soon as the first k-range of the weight gather lands.

_CACHE = {}


def _p(s, g, h, m):
    return ((s * G + g) * NH + h) * MCH + m


def _ap(tile_ap, off, dims):
    """Raw strided AP on a tile's underlying tensor. dims=[(step,count),...]
    in elements; for SBUF the partition stride is ap[0][0] of the base AP."""
    import bass_rust

    return bass_rust.AP(tile_ap.tensor, tile_ap.offset + off, [list(d) for d in dims])


def build_module():
    if "nc" in _CACHE:
        return _CACHE["nc"]
    import concourse.bacc as bacc
    import concourse.mybir as mybir
    import concourse.tile as tile

    dt = mybir.dt
    AF = mybir.ActivationFunctionType
    ALU = mybir.AluOpType

    nc = bacc.Bacc("TRN2", debug=False, num_devices=N_CORES)

    # ---- DRAM I/O ----------------------------------------------------------
    xin = nc.dram_tensor("xin", [S, CIN, HW], dt.float32, kind="ExternalInput")
    w1t = nc.dram_tensor("w1t", [2, 128, CMID], dt.bfloat16, kind="ExternalInput")
    b1 = nc.dram_tensor("b1", [CMID, 1], dt.float32, kind="ExternalInput")
    c1t = nc.dram_tensor("c1t", [CMID, R], dt.bfloat16, kind="ExternalInput")
    bi = nc.dram_tensor("bi", [R, 1], dt.float32, kind="ExternalInput")
    c2t = nc.dram_tensor("c2t", [R, G * KK], dt.bfloat16, kind="ExternalInput")
    b2ca = nc.dram_tensor("b2ca", [128, 1], dt.float32, kind="ExternalInput")
    b2cb = nc.dram_tensor("b2cb", [68, 1], dt.float32, kind="ExternalInput")
    s2v = nc.dram_tensor("s2v", [CMID, 1], dt.float32, kind="ExternalInput")
    b2v = nc.dram_tensor("b2v", [CMID, 1], dt.float32, kind="ExternalInput")
    w3t = nc.dram_tensor("w3t", [2, CMID, 128], dt.bfloat16, kind="ExternalInput")
    b3 = nc.dram_tensor("b3", [128, 2], dt.float32, kind="ExternalInput")
    ident = nc.dram_tensor("ident", [128, 128], dt.bfloat16, kind="ExternalInput")
    out = nc.dram_tensor("out", [S, CIN, HW], dt.bfloat16, kind="ExternalOutput")

    with tile.TileContext(nc) as tc:
        with (
            tc.tile_pool(name="consts", bufs=1) as cpool,
            tc.tile_pool(name="big", bufs=1) as bpool,
            tc.tile_pool(name="zst", bufs=4) as zpool,
            tc.tile_pool(name="rst", bufs=2) as rpool,
            tc.tile_pool(name="ob", bufs=2) as opool,
            tc.tile_pool(name="dstage", bufs=1, space="DRAM") as dpool,
        ):
            # ---- constants -> SBUF ----------------------------------------
            w1t_sb = cpool.tile([128, 2 * CMID], dt.bfloat16, tag="w1t")
            nc.sync.dma_start(
                out=w1t_sb[:, :].rearrange("p (k c) -> p k c", k=2),
                in_=w1t.ap().rearrange("k p c -> p k c"),
            )
            b1_sb = cpool.tile([CMID, 1], dt.float32, tag="b1")
            nc.sync.dma_start(out=b1_sb[:, :], in_=b1.ap())
            c1t_sb = cpool.tile([CMID, R], dt.bfloat16, tag="c1t")
            nc.sync.dma_start(out=c1t_sb[:, :], in_=c1t.ap())
            bi_sb = cpool.tile([R, 1], dt.float32, tag="bi")
            nc.sync.dma_start(out=bi_sb[:, :], in_=bi.ap())
            c2t_sb = cpool.tile([R, G * KK], dt.bfloat16, tag="c2t")
            nc.sync.dma_start(out=c2t_sb[:, :], in_=c2t.ap())
            b2ca_sb = cpool.tile([128, 1], dt.float32, tag="b2ca")
            nc.sync.dma_start(out=b2ca_sb[:, :], in_=b2ca.ap())
            b2cb_sb = cpool.tile([68, 1], dt.float32, tag="b2cb")
            nc.sync.dma_start(out=b2cb_sb[:, :], in_=b2cb.ap())
            s2v_sb = cpool.tile([CMID, 1], dt.float32, tag="s2v")
            nc.sync.dma_start(out=s2v_sb[:, :], in_=s2v.ap())
            b2v_sb = cpool.tile([CMID, 1], dt.float32, tag="b2v")
            nc.sync.dma_start(out=b2v_sb[:, :], in_=b2v.ap())
            w3t_sb = cpool.tile([CMID, 2 * 128], dt.bfloat16, tag="w3t")
            nc.sync.dma_start(
                out=w3t_sb[:, :].rearrange("p (k c) -> p k c", k=2),
                in_=w3t.ap().rearrange("k p c -> p k c"),
            )
            b3_sb = cpool.tile([128, 2], dt.float32, tag="b3")
            nc.sync.dma_start(out=b3_sb[:, :], in_=b3.ap())
            id_sb = cpool.tile([128, 128], dt.bfloat16, tag="ident")
            nc.sync.dma_start(out=id_sb[:, :], in_=ident.ap())

            # ---- big SBUF tiles -------------------------------------------
            xbf = bpool.tile([128, S * 2 * HW], dt.bfloat16, tag="xbf")
            # sized for its out2 reuse (62-padded layout needs S*MCH*NWP=6944)
            out1 = bpool.tile([CMID, S * MCH * NWP], dt.bfloat16, tag="out1")
            w2a = bpool.tile([128, S * MCH * NWP], dt.bfloat16, tag="w2a")
            w2b = bpool.tile([68, S * MCH * NWP], dt.bfloat16, tag="w2b")
            xu = bpool.tile([NP, XUF], dt.bfloat16, tag="xu")
            xh = bpool.tile([NP, XHF], dt.bfloat16, tag="xh")
            xh2 = bpool.tile([NP, XHF], dt.bfloat16, tag="xh2")
            w2t = bpool.tile([NP, W2F], dt.bfloat16, tag="w2t")
            acc2 = bpool.tile([NP, 2 * ACCF], dt.bfloat16, tag="acc2")
            tmp2 = [
                bpool.tile([NP, 2 * ACCF], dt.bfloat16, tag=f"tmp2_{i}", name=f"tmp2_{i}")
                for i in range(2)
            ]
            out2 = out1  # out1 is dead after the xu gathers; reuse for out2
            zt = bpool.tile([CMID, PAD], dt.bfloat16, tag="zt")
            zst2 = [
                bpool.tile([R, NWP], dt.bfloat16, tag=f"zst{i}", name=f"zst{i}")
                for i in range(2)
            ]

            xbf_v = xbf[:, :].rearrange("p (s k f) -> p s k f", s=S, k=2)
            xu_v = xu[:, :].rearrange("p (c r w) -> p c r w", c=C8, r=HR, w=W)
            xh_v = xh[:, :].rearrange("p (c r w) -> p c r w", c=C8, r=HR, w=WP)
            xh2_v = xh2[:, :].rearrange("p (c r w) -> p c r w", c=C8, r=HR, w=WP)
            # flat views for the long-run tap ops
            xh_f = xh[:, :].rearrange("p (c f) -> p c f", c=C8)
            xh2_f = xh2[:, :].rearrange("p (c f) -> p c f", c=C8)
            P_XH = xh[:, :].ap[0][0]
            P_XH2 = xh2[:, :].ap[0][0]
            P_A2 = acc2[:, :].ap[0][0]
            P_T2 = [t[:, :].ap[0][0] for t in tmp2]

            # ---- DRAM staging ---------------------------------------------
            # w2d is (s,m)-major so the w2t gather per partition is one
            # contiguous multi-KB run (DRAM-sequential, not 111KB strides).
            out1d = dpool.tile([CMID, S * SPX], dt.bfloat16, tag="out1d")
            w2d = dpool.tile([S * MCH, G * KK * NWP], dt.bfloat16, tag="w2d")
            accd = dpool.tile([CMID, S * MCH * NWP], dt.bfloat16, tag="accd")

            # ---- memzeros (pads for halo tensors) -------------------------
            # all on DVE so the ACT queue is free for the conv1 evacuations.
            # Small FE-critical tiles (zt feeds the out1d margins, zst2 gates
            # the inv_c1 evacs) go FIRST; the big halo tiles are only needed
            # once the xu gathers land.  acc2 needs no memset: every cell that
            # is ever read is written by the first tap pair or the PSUM evacs
            # (pad tails carry junk that never reaches valid output columns).
            nc.vector.memset(zt[:, :], 0.0)
            for z in zst2:
                nc.vector.memset(z[:, :], 0.0)
            nc.vector.memset(xu[:, :], 0.0)
            nc.vector.memset(xh[:, :], 0.0)
            nc.vector.memset(xh2[:, :], 0.0)

            # zero margins of out1d so halo gathers read zeros off the edges
            for s in range(S):
                nc.sync.dma_start(
                    out=out1d[:, s * SPX : s * SPX + PAD], in_=zt[:, :]
                )
                nc.sync.dma_start(
                    out=out1d[:, s * SPX + PAD + HW : (s + 1) * SPX], in_=zt[:, :]
                )

            # ---- x load (f32 -> bf16 cast; SWDGE on gpsimd) ---------------
            # split in half-sample pieces so conv1 can start on the first
            # chunks while the rest streams in
            HWH = HW // 2
            for s in range(S):
                for hh in range(2):
                    for kc in range(2):
                        nc.gpsimd.dma_start(
                            out=xbf_v[:, s, kc, hh * HWH : (hh + 1) * HWH],
                            in_=xin.ap()[
                                s, kc * 128 : (kc + 1) * 128, hh * HWH : (hh + 1) * HWH
                            ],
                        )

            # ---- per-sample front end: conv1 / inv_c1 / inv_c2 ------------
            w1t_v = w1t_sb[:, :].rearrange("p (k c) -> p k c", k=2)
            o1d_ap = out1d[:, :]
            w2d_ap = w2d[:, :]
            xu_ap = xu[:, :]
            w2t_ap = w2t[:, :]
            D1 = o1d_ap.ap[0][0]
            D2 = w2d_ap.ap[0][0]
            P_XU = xu_ap.ap[0][0]
            P_W2T = w2t_ap.ap[0][0]

            pfe_cm = tc.tile_pool(name="psum_fe", bufs=2, space="PSUM")
            pfe = pfe_cm.__enter__()
            for s in range(S):
                for n0 in range(0, NCH, 2):
                    # conv1 (256->64) + BN1 + ReLU   [ACT evac]
                    # 2-chunk batches per stationary operand: LDWEIGHTS for
                    # w1t[kc] loads once per pair instead of per chunk.
                    pss = []
                    for kc in range(2):
                        for dn in range(2):
                            n = n0 + dn
                            if kc == 0:
                                pss.append(
                                    pfe.tile(
                                        [128, NW], dt.float32, tag="ps", bufs=3,
                                        name=f"ps_{s}_{n}",
                                    )
                                )
                            nc.tensor.matmul(
                                pss[dn][:CMID, :],
                                w1t_v[:, kc, :],
                                xbf_v[:, s, kc, n * NW : (n + 1) * NW],
                                start=(kc == 0),
                                stop=(kc == 1),
                            )
                    for dn in range(2):
                        n = n0 + dn
                        sl = slice(s * HW + n * NW, s * HW + (n + 1) * NW)
                        nc.scalar.activation(
                            out1[:, sl], pss[dn][:CMID, :], AF.Relu, bias=b1_sb[:, 0:1]
                        )
                    for dn in range(2):
                        n = n0 + dn
                        sl = slice(s * HW + n * NW, s * HW + (n + 1) * NW)
                        # inv_c1 (64->16) + BN + ReLU    [ACT evac]
                        # zst is row-padded to 62-wide rows; the pad columns
                        # keep stale (finite) values which flow through inv_c2
                        # into pad weight columns that only multiply zeros.
                        ps1 = pfe.tile([128, NW], dt.float32, tag="ps", bufs=3)
                        nc.tensor.matmul(
                            ps1[:R, :], c1t_sb[:, :], out1[:, sl], start=True, stop=True
                        )
                        zst = zst2[n % 2]
                        zst_v = zst[:, :].rearrange("p (r w) -> p r w", r=RH, w=WP)
                        nc.scalar.activation(
                            zst_v[:, :, 0:W],
                            ps1[:R, :].rearrange("p (r w) -> p r w", r=RH, w=W),
                            AF.Relu,
                            bias=bi_sb[:, 0:1],
                        )
                        # inv_c2 (16->196) + bias        [DVE + ACT evacs]
                        psa = pfe.tile([128, NWP], dt.float32, tag="psw", bufs=4)
                        psb = pfe.tile([128, NWP], dt.float32, tag="psw", bufs=4)
                        nc.tensor.matmul(
                            psa[:, :], c2t_sb[:, 0:128], zst[:, :], start=True, stop=True
                        )
                        nc.tensor.matmul(
                            psb[:68, :], c2t_sb[:, 128:196], zst[:, :],
                            start=True, stop=True,
                        )
                        nsl = slice((s * MCH + n) * NWP, (s * MCH + n + 1) * NWP)
                        nc.vector.tensor_scalar(
                            w2a[:, nsl], psa[:, :], b2ca_sb[:, 0:1], None, op0=ALU.add
                        )
                        nc.scalar.activation(
                            w2b[:, nsl], psb[:68, :], AF.Identity, bias=b2cb_sb[:, 0:1]
                        )

                    # stage out1 through DRAM as each 2-chunk batch finishes
                    nc.sync.dma_start(
                        out=out1d[
                            :,
                            s * SPX + PAD + n0 * NW : s * SPX + PAD + (n0 + 2) * NW,
                        ],
                        in_=out1[:, s * HW + n0 * NW : s * HW + (n0 + 2) * NW],
                    )

                # w2 -> (s,m)-major DRAM per sample (dst runs gk-seq 868B)
                nc.sync.dma_start(
                    out=_ap(w2d_ap, s * MCH * D2, [(NWP, 128), (D2, MCH), (1, NWP)]),
                    in_=_ap(
                        w2a[:, :],
                        s * MCH * NWP,
                        [(w2a[:, :].ap[0][0], 128), (NWP, MCH), (1, NWP)],
                    ),
                )
                nc.sync.dma_start(
                    out=_ap(
                        w2d_ap,
                        128 * NWP + s * MCH * D2,
                        [(NWP, 68), (D2, MCH), (1, NWP)],
                    ),
                    in_=_ap(
                        w2b[:, :],
                        s * MCH * NWP,
                        [(w2b[:, :].ap[0][0], 68), (NWP, MCH), (1, NWP)],
                    ),
                )

                # first w2t k-range for this sample right behind its w2d so
                # the first tap pair-group can start as soon as possible
                kh0, kn = 0, 14
                for g in range(G):
                    for h in range(NH):
                        nc.sync.dma_start(
                            out=_ap(
                                w2t_ap,
                                _p(s, g, h, 0) * P_W2T + kh0 * NWP,
                                [(P_W2T, MCH), (1, kn * NWP)],
                            ),
                            in_=_ap(
                                w2d_ap,
                                s * MCH * D2 + (g * KK + kh0) * NWP,
                                [(D2, MCH), (1, kn * NWP)],
                            ),
                        )

                # xu gathers on the SWDGE queue (16-engine, parallel with SP);
                # c-outer/m-inner so source reads walk DRAM monotonically
                for g in range(G):
                    for h in range(NH):
                        nc.gpsimd.dma_start(
                            out=_ap(
                                xu_ap,
                                _p(s, g, h, 0) * P_XU,
                                [(P_XU, MCH), (HR * W, C8), (1, 13 * W)],
                            ),
                            in_=_ap(
                                o1d_ap,
                                (g * GC + h * C8) * D1 + s * SPX,
                                [(NW, MCH), (D1, C8), (1, 13 * W)],
                            ),
                        )
            pfe_cm.__exit__(None, None, None)

            # remaining w2t k-ranges (consumed progressively by the taps).
            # On the SWDGE queue BEHIND the xu gathers: their 4MB would
            # otherwise saturate the DMA engines exactly when the xu(s1) ->
            # xh chain (the critical path into the first tap) needs them.
            for kh0, kn in ((14, 14), (28, 14), (42, 7)):
                for s in range(S):
                    for g in range(G):
                        for h in range(NH):
                            nc.gpsimd.dma_start(
                                out=_ap(
                                    w2t_ap,
                                    _p(s, g, h, 0) * P_W2T + kh0 * NWP,
                                    [(P_W2T, MCH), (1, kn * NWP)],
                                ),
                                in_=_ap(
                                    w2d_ap,
                                    s * MCH * D2 + (g * KK + kh0) * NWP,
                                    [(D2, MCH), (1, kn * NWP)],
                                ),
                            )

            # ---- halo expansion: XU -> XH / XH2, both on DVE (4x copies,
            # ~1.8us each; ACT would take 6.7us and sits on the critical
            # path into the first tap) ----
            for s in range(S):
                pl = slice(s * 64, (s + 1) * 64)
                nc.vector.tensor_copy(xh_v[pl, :, :, 3 : 3 + W], xu_v[pl])
                nc.vector.tensor_copy(xh2_v[pl, :, :, 4 : 4 + W], xu_v[pl])

            # ---- involution taps: kh-pairs merged into single DVE ops ----
            # Pair (kh, kh+1) for kh in {0,2,4} plus single kh=6, per kw.
            # Each pair op has a k2 dim (stride 62 in xh, 7*NWP in w2t,
            # ACCF in acc2) -> two partial sums in acc2's slots; one final
            # slot-add after all taps.
            # Channel split: channels 0..CP-1 of each (g,h) accumulate on the
            # PE (identity-stationary matmuls into PSUM, both tap slots into
            # one bank so no slot fold is needed); channels CP..7 accumulate
            # on DVE as before.  DVE multiplies for all 8 channels.
            ptap_cm = tc.tile_pool(name="psum_tap", bufs=1, space="PSUM")
            ptap = ptap_cm.__enter__()
            pacc = [
                ptap.tile(
                    [128, NWP], dt.float32, tag=f"pacc{c}", bufs=1, name=f"pacc{c}"
                )
                for c in range(CP)
            ]
            w2t_raw = w2t[:, :]
            first = True
            ti = 0
            nmm = [0] * CP
            for kh in (0, 2, 4, 6):
                npair = 2 if kh < 6 else 1
                for kw in (1, 3, 5, 0, 2, 4, 6):
                    if kw % 2 == 0:
                        srcap, psrc, base = xh[:, :], P_XH, kh * WP + kw
                    else:
                        srcap, psrc, base = xh2[:, :], P_XH2, kh * WP + kw + 1
                    k = kh * KS + kw
                    t = ti % 2
                    ti += 1
                    if first:
                        # first pair: DVE channels multiply straight into
                        # acc2; PE channels into tmp for the matmul chain
                        nc.vector.tensor_mul(
                            _ap(
                                acc2[:, :], CP * NWP,
                                [(P_A2, NP), (ACCF, npair), (NWP, CV), (1, RUN)],
                            ),
                            _ap(
                                srcap, base + CP * 868,
                                [(psrc, NP), (WP, npair), (868, CV), (1, RUN)],
                            ),
                            _ap(
                                w2t_raw, k * NWP,
                                [(P_W2T, NP), (KS * NWP, npair), (0, CV), (1, RUN)],
                            ),
                        )
                        nc.vector.tensor_mul(
                            _ap(
                                tmp2[t][:, :], 0,
                                [(P_T2[t], NP), (ACCF, npair), (NWP, CP), (1, RUN)],
                            ),
                            _ap(
                                srcap, base,
                                [(psrc, NP), (WP, npair), (868, CP), (1, RUN)],
                            ),
                            _ap(
                                w2t_raw, k * NWP,
                                [(P_W2T, NP), (KS * NWP, npair), (0, CP), (1, RUN)],
                            ),
                        )
                        first = False
                    else:
                        nc.vector.tensor_mul(
                            _ap(
                                tmp2[t][:, :], 0,
                                [(P_T2[t], NP), (ACCF, npair), (NWP, C8), (1, RUN)],
                            ),
                            _ap(
                                srcap, base,
                                [(psrc, NP), (WP, npair), (868, C8), (1, RUN)],
                            ),
                            _ap(
                                w2t_raw, k * NWP,
                                [(P_W2T, NP), (KS * NWP, npair), (0, C8), (1, RUN)],
                            ),
                        )
                        nc.vector.tensor_add(
                            _ap(
                                acc2[:, :], CP * NWP,
                                [(P_A2, NP), (ACCF, npair), (NWP, CV), (1, RUN)],
                            ),
                            _ap(
                                acc2[:, :], CP * NWP,
                                [(P_A2, NP), (ACCF, npair), (NWP, CV), (1, RUN)],
                            ),
                            _ap(
                                tmp2[t][:, :], CP * NWP,
                                [(P_T2[t], NP), (ACCF, npair), (NWP, CV), (1, RUN)],
                            ),
                        )
                    # PE accumulation for channels 0..CP-1 (both slots)
                    for sp in range(npair):
                        for c in range(CP):
                            nc.tensor.matmul(
                                pacc[c][:, :],
                                id_sb[:, :],
                                tmp2[t][:, sp * ACCF + c * NWP : sp * ACCF + (c + 1) * NWP],
                                start=(nmm[c] == 0),
                                stop=(nmm[c] == KK - 1),
                                skip_group_check=True,
                            )
                            nmm[c] += 1
            # fold slot1 into slot0 (DVE channels only)
            s0 = _ap(acc2[:, :], CP * NWP, [(P_A2, NP), (NWP, CV), (1, NWP)])
            s1 = _ap(acc2[:, :], ACCF + CP * NWP, [(P_A2, NP), (NWP, CV), (1, NWP)])
            nc.vector.tensor_add(s0, s0, s1)
            # evacuate the PE accumulators into acc2 slot0 (split ACT/DVE)
            for c in range(CP):
                if c % 2 == 0:
                    nc.scalar.copy(acc2[:, c * NWP : (c + 1) * NWP], pacc[c][:, :])
                else:
                    nc.vector.tensor_copy(
                        acc2[:, c * NWP : (c + 1) * NWP], pacc[c][:, :]
                    )
            ptap_cm.__exit__(None, None, None)

            # ---- acc -> DRAM channel-major scatter (62-padded throughout;
            # the pad columns carry exact zeros / ignorable junk) ------------
            # per sample: scatter split across both queues, then that
            # sample's out2 readback immediately, so BN2(s0) can start while
            # sample 1 is still being scattered
            acd_ap = accd[:, :]
            D3 = acd_ap.ap[0][0]
            acc_ap = acc2[:, :]
            P_AC = P_A2
            SMW = MCH * NWP  # 3472 padded pixels per sample
            out2v = out1[:, 0 : S * SMW]
            for s in range(S):
                for g in range(G):
                    for h in range(NH):
                        eng = nc.sync
                        eng.dma_start(
                            out=_ap(
                                acd_ap,
                                (g * GC + h * C8) * D3 + s * MCH * NWP,
                                [(NWP, MCH), (D3, C8), (1, NWP)],
                            ),
                            in_=_ap(
                                acc_ap,
                                _p(s, g, h, 0) * P_AC,
                                [(P_AC, MCH), (NWP, C8), (1, NWP)],
                            ),
                        )
                # out2 reuses the (dead) out1 tile, 62-padded layout;
                # split in halves so BN2 starts on the first half sooner
                HSM = SMW // 2
                for hh in range(2):
                    nc.sync.dma_start(
                        out=out2v[:, s * SMW + hh * HSM : s * SMW + (hh + 1) * HSM],
                        in_=accd[:, s * SMW + hh * HSM : s * SMW + (hh + 1) * HSM],
                    )

            # ---- BN2 + ReLU; conv3 + BN3 + residual + ReLU ----------------
            pbe_cm = tc.tile_pool(name="psum_be", bufs=2, space="PSUM")
            pbe = pbe_cm.__enter__()
            w3t_v = w3t_sb[:, :].rearrange("p (k c) -> p k c", k=2)
            eng_flip = 0
            for s in range(S):
                obufs = [
                    opool.tile([128, HW], dt.bfloat16, tag="ob", name=f"ob{s}_{i}")
                    for i in range(2)
                ]
                for q in range(4):
                    rst = rpool.tile([CMID, 2 * NWP], dt.bfloat16, tag="rst")
                    nc.scalar.activation(
                        rst[:, :],
                        out2v[:, s * SMW + q * 2 * NWP : s * SMW + (q + 1) * 2 * NWP],
                        AF.Relu,
                        bias=b2v_sb[:, 0:1],
                        scale=s2v_sb[:, 0:1],
                    )
                    rst_v = rst[:, :].rearrange(
                        "p (m r w) -> p m r w", m=2, r=RH, w=WP
                    )
                    # all 4 (oc, hf) PSUM tiles of this q at once: w3 runs as
                    # 2-chunk batches per oc stationary, then one identity
                    # pass over all 4 (3 LDWEIGHTS per q instead of 8), with
                    # bufs=8 so the next q's matmuls overlap these evacs
                    pss = [
                        pbe.tile(
                            [128, NW], dt.float32, tag="ps", bufs=8,
                            name=f"ps3_{s}_{q}_{j}",
                        )
                        for j in range(4)
                    ]
                    for oc in range(2):
                        for hf in range(2):
                            nc.tensor.matmul(
                                pss[oc * 2 + hf][:, :],
                                w3t_v[:, oc, :],
                                rst_v[:, hf, :, 0:W],
                                start=True,
                                stop=False,
                            )
                    for oc in range(2):
                        for hf in range(2):
                            nx = (q * 2 + hf) * NW
                            nc.tensor.matmul(
                                pss[oc * 2 + hf][:, :],
                                id_sb[:, :],
                                xbf_v[:, s, oc, nx : nx + NW],
                                start=False,
                                stop=True,
                            )
                    for oc in range(2):
                        for hf in range(2):
                            nx = (q * 2 + hf) * NW
                            dst = obufs[oc][:, nx : nx + NW]
                            if eng_flip % 2 == 0:
                                nc.vector.tensor_scalar(
                                    dst,
                                    pss[oc * 2 + hf][:, :],
                                    b3_sb[:, oc : oc + 1],
                                    0.0,
                                    op0=ALU.add,
                                    op1=ALU.max,
                                )
                            else:
                                nc.scalar.activation(
                                    dst, pss[oc * 2 + hf][:, :], AF.Relu,
                                    bias=b3_sb[:, oc : oc + 1],
                                )
                            eng_flip += 1
                for oc in range(2):
                    nc.sync.dma_start(
                        out=out.ap()[s, oc * 128 : (oc + 1) * 128, :],
                        in_=obufs[oc][:, :],
                    )
            pbe_cm.__exit__(None, None, None)

    nc.compile()
    _CACHE["nc"] = nc
    return nc


def _f32(a):
    return np.ascontiguousarray(a, dtype=np.float32)


def prep_weights(inputs):
    """Host-side folding of BN scales into conv weights; bf16 casts."""
    f = inputs
    s1 = f["bn1_g"] / np.sqrt(f["bn1_v"] + EPS)
    b1_eff = f["bn1_b"] - f["bn1_m"] * s1
    w1t_eff = (_f32(f["conv1_w"]) * s1[:, None]).T          # [256, 64]

    si = f["inv_bn_g"] / np.sqrt(f["inv_bn_v"] + EPS)
    bi_eff = f["inv_bn_b"] - f["inv_bn_m"] * si
    c1t_eff = (_f32(f["inv_c1_w"]) * si[:, None]).T         # [64, 16]

    c2t_eff = _f32(f["inv_c2_w"]).T                         # [16, 196]
    b2c = _f32(f["inv_c2_b"])

    s2 = f["bn2_g"] / np.sqrt(f["bn2_v"] + EPS)
    b2n = f["bn2_b"] - f["bn2_m"] * s2

    s3 = f["bn3_g"] / np.sqrt(f["bn3_v"] + EPS)
    b3_eff = f["bn3_b"] - f["bn3_m"] * s3
    w3t_eff = (_f32(f["conv3_w"]) * s3[:, None]).T          # [64, 256]

    d = {}
    d["w1t"] = np.ascontiguousarray(w1t_eff.reshape(2, 128, CMID).astype(BF16))
    d["b1"] = _f32(b1_eff)[:, None]
    d["c1t"] = np.ascontiguousarray(c1t_eff.astype(BF16))
    d["bi"] = _f32(bi_eff)[:, None]
    d["c2t"] = np.ascontiguousarray(c2t_eff.astype(BF16))
    d["b2ca"] = _f32(b2c[0:128])[:, None]
    d["b2cb"] = _f32(b2c[128:196])[:, None]
    d["s2v"] = _f32(s2)[:, None]
    d["b2v"] = _f32(b2n)[:, None]
    d["w3t"] = np.ascontiguousarray(
        w3t_eff.reshape(CMID, 2, 128).transpose(1, 0, 2).astype(BF16)
    )
    d["b3"] = _f32(b3_eff.reshape(2, 128).T)
    d["ident"] = np.ascontiguousarray(np.eye(128, dtype=np.float32).astype(BF16))
    return d


def make_in_maps(inputs):
    prep = prep_weights(inputs)
    x = _f32(inputs["x"]).reshape(16, CIN, HW)
    in_maps = []
    for i in range(N_CORES):
        m = dict(prep)
        m["xin"] = np.ascontiguousarray(x[S * i : S * i + S])
        in_maps.append(m)
    return in_maps


def kernel(**inputs):
    from concourse.bass_utils import run_bass_kernel_spmd

    nc = build_module()
    in_maps = make_in_maps(inputs)
    res = run_bass_kernel_spmd(nc, in_maps, core_ids=list(range(N_CORES)))
    outs = [
        np.asarray(res.results[i]["out"], dtype=np.float32).reshape(S, CIN, H, W)
        for i in range(N_CORES)
    ]
    return np.concatenate(outs, axis=0).astype(np.float32)

